# revision 2
# baseline (speedup 1.0000x reference)
"""Trainium2 Bass kernel v10 for nn_Attention_Layer (B=4, S=2048, D=1024, fp32).

Sharding: 8 cores = 4 batches x 2 query-halves. Each core computes K^T for the
whole sequence (duplicated within the batch pair) and Q/scores/output for its
1024-query half.

The V projection is reassociated away:
    y = P @ (X Wv + bv) / Z = (P^T X)^T Wv / Z + bv
computing T^T = X^T P (cost of the old attn@V) then y = T^T.T @ Wv over only
this core's 1024 queries — saves the duplicated V projection (~11% of PE
work). The softmax denominator Z is computed with a single ones-column
stationary pass and folded into the output eviction as a per-partition ACT
scale.

All matmul operands are bf16 (216 ns/MM at full clock); PSUM accumulation is
fp32; exp runs on ACT in fp32. Wk/Wq stay fully SBUF-resident; DMAs are
ordered so the first matmul only gates on ~1.3MB.
"""

import numpy as np

import concourse.mybir as mybir
import concourse.tile as tile
from concourse import bacc
from concourse.bass_utils import run_bass_kernel_spmd

B, S, D = 4, 2048, 1024
P = 128
HALF = S // 2            # queries per core
EO = D // P              # 8 output-feature tiles
DO = D // P              # 8 contraction tiles
KO = S // P              # 16 key tiles
QT = HALF // P           # 8 query tiles per core
SCALE = 1.0 / np.sqrt(D)

F32 = mybir.dt.float32
BF16 = mybir.dt.bfloat16


def build_nc():
    nc = bacc.Bacc("TRN2", target_bir_lowering=False)

    xT = nc.dram_tensor("xT", [D, S], BF16, kind="ExternalInput")      # [d, s]
    xN = nc.dram_tensor("xN", [S, D], BF16, kind="ExternalInput")      # [s, d]
    Wk = nc.dram_tensor("Wk", [EO, P, DO * P], BF16, kind="ExternalInput")
    Wq = nc.dram_tensor("Wq", [EO, P, DO * P], BF16, kind="ExternalInput")
    Wv = nc.dram_tensor("Wv", [D, D], BF16, kind="ExternalInput")      # [d, e]
    bkT = nc.dram_tensor("bkT", [P, EO], F32, kind="ExternalInput")
    bqT = nc.dram_tensor("bqT", [P, EO], F32, kind="ExternalInput")
    bv = nc.dram_tensor("bv", [P, D], F32, kind="ExternalInput")       # replicated
    y = nc.dram_tensor("y", [HALF, D], F32, kind="ExternalOutput")

    xTr = xT.ap().rearrange("(do p) s -> p do s", p=P)
    xNr = xN.ap().rearrange("(ko p) d -> p ko d", p=P)
    Wvr = Wv.ap().rearrange("(dt p) e -> p dt e", p=P)

    with tile.TileContext(nc) as tc:
        with (
            tc.tile_pool(name="kt", bufs=1) as k_pool,         # 32KB/part
            tc.tile_pool(name="qt", bufs=1) as q_pool,         # 16KB
            tc.tile_pool(name="small", bufs=1) as small_pool,
            tc.tile_pool(name="ps", bufs=5, space="PSUM") as ps_pool,
            tc.tile_pool(name="zp", bufs=1, space="PSUM") as z_pool,
            tc.tile_pool(name="dram", bufs=1, space="DRAM") as dram_pool,
        ):
            bk_sb = small_pool.tile([P, EO], F32, tag="bk")
            bq_sb = small_pool.tile([P, EO], F32, tag="bq")
            ones_sb = small_pool.tile([P, 1], BF16, tag="ones")
            nc.vector.memset(ones_sb[:], 1.0)

            k_sb = k_pool.tile([P, EO, S], BF16, tag="kt")
            q_sb = q_pool.tile([P, EO, HALF], BF16, tag="qt")

            # keep the PE's HAM clock gate warm through the initial DMA wait
            warm_ps = z_pool.tile([1, 8], F32, tag="warm")
            for _ in range(160):
                nc.tensor.matmul(
                    warm_ps[:, 0:1], ones_sb[:], ones_sb[:],
                    start=True, stop=True,
                )

            # ---- scope 1: K/Q projections -------------------------------
            with (
                tc.tile_pool(name="xt", bufs=1) as xt_pool,    # 32KB
                tc.tile_pool(name="wk", bufs=1) as wk_pool,    # 16KB
                tc.tile_pool(name="wq", bufs=1) as wq_pool,    # 16KB
            ):
                xt_sb = xt_pool.tile([P, DO, S], BF16, tag="xt")
                wk_sb = wk_pool.tile([P, EO, DO, P], BF16, tag="wk")
                wq_sb = wq_pool.tile([P, EO, DO, P], BF16, tag="wq")

                def w_load(dst, Wt):
                    for eo in range(EO):
                        nc.sync.dma_start(
                            dst[:, eo], Wt[eo].unsqueeze(0).rearrange(
                                "o p (do e) -> (o p) do e", do=DO
                            ),
                        )

                # DMA priority order: first s-quarter (gates phase A1a), all
                # of Wk, rest of first half, biases, Wq, second half.
                for do in range(DO):
                    nc.sync.dma_start(xt_sb[:, do, 0:512], xTr[:, do, 0:512])
                w_load(wk_sb, Wk)
                for do in range(DO):
                    nc.sync.dma_start(xt_sb[:, do, 512:HALF], xTr[:, do, 512:HALF])
                nc.sync.dma_start(bk_sb[:], bkT[:, :])
                nc.sync.dma_start(bq_sb[:], bqT[:, :])
                w_load(wq_sb, Wq)
                for do in range(DO):
                    nc.sync.dma_start(xt_sb[:, do, HALF:S], xTr[:, do, HALF:S])

                def proj(w_res, b_sb, dst_sb, s0, s1):
                    nblk = (s1 - s0) // 512
                    for eo in range(EO):
                        pss = []
                        for j in range(nblk):
                            psj = ps_pool.tile([P, 512], F32, tag="ps")
                            pss.append(psj)
                        for do in range(DO):
                            for j in range(nblk):
                                nc.tensor.matmul(
                                    pss[j][:],
                                    w_res[:, eo, do],
                                    xt_sb[:, do, s0 + j * 512 : s0 + (j + 1) * 512],
                                    start=(do == 0), stop=(do == DO - 1),
                                )
                        for j in range(nblk):
                            nc.vector.tensor_scalar_add(
                                dst_sb[:, eo, s0 + j * 512 : s0 + (j + 1) * 512],
                                pss[j][:],
                                b_sb[:, eo : eo + 1],
                            )

                proj(wk_sb, bk_sb, k_sb, 0, 512)      # K^T keys 0:512
                proj(wk_sb, bk_sb, k_sb, 512, HALF)   # K^T keys 512:1024
                proj(wq_sb, bq_sb, q_sb, 0, HALF)     # Q^T own queries
                proj(wk_sb, bk_sb, k_sb, HALF, S)     # K^T keys 1024:2048

            # ---- scope 2: scores / T^T / y ------------------------------
            with (
                tc.tile_pool(name="pt", bufs=1) as p_pool,     # 32KB
                tc.tile_pool(name="tt", bufs=1) as t_pool,     # 16KB
                tc.tile_pool(name="wv", bufs=1) as wv_pool,    # 16KB
                tc.tile_pool(name="xn", bufs=1) as xn_pool,    # 32KB
                tc.tile_pool(name="outp", bufs=2) as out_pool,  # 4KB
            ):
                zrow_sb = small_pool.tile([1, HALF], F32, tag="zrow")
                rz_sb = small_pool.tile([P, QT], F32, tag="rz")
                p_sb = p_pool.tile([P, KO, HALF], BF16, tag="pt")
                tT_sb = t_pool.tile([P, DO, HALF], BF16, tag="tt")
                bv_sb = small_pool.tile([P, D], F32, tag="bv")

                xn_sb = xn_pool.tile([P, KO, D], BF16, tag="xn")
                for ko in range(KO):
                    nc.sync.dma_start(xn_sb[:, ko, :], xNr[:, ko, :])
                wv_sb = wv_pool.tile([P, DO, D], BF16, tag="wv")
                for dt in range(DO):
                    nc.sync.dma_start(wv_sb[:, dt, :], Wvr[:, dt, :])
                nc.sync.dma_start(bv_sb[:], bv[:, :])

                # scores^T + exp: contraction over features (eo)
                for ktl in range(KO):
                    ps0 = ps_pool.tile([P, 512], F32, tag="ps")
                    ps1 = ps_pool.tile([P, 512], F32, tag="ps")
                    for eo in range(EO):
                        kt_ap = k_sb[:, eo, ktl * P : (ktl + 1) * P]
                        nc.tensor.matmul(
                            ps0[:], kt_ap, q_sb[:, eo, 0:512],
                            start=(eo == 0), stop=(eo == EO - 1),
                        )
                        nc.tensor.matmul(
                            ps1[:], kt_ap, q_sb[:, eo, 512:1024],
                            start=(eo == 0), stop=(eo == EO - 1),
                        )
                    nc.scalar.activation(
                        p_sb[:, ktl, 0:512], ps0[:],
                        mybir.ActivationFunctionType.Exp, scale=float(SCALE),
                    )
                    nc.scalar.activation(
                        p_sb[:, ktl, 512:1024], ps1[:],
                        mybir.ActivationFunctionType.Exp, scale=float(SCALE),
                    )

                # Z^T[1, q] = ones^T @ P (single ldweights for the phase)
                z0 = z_pool.tile([1, 512], F32, tag="z0")
                z1 = z_pool.tile([1, 512], F32, tag="z1")
                for ko in range(KO):
                    nc.tensor.matmul(
                        z0[:], ones_sb[:], p_sb[:, ko, 0:512],
                        start=(ko == 0), stop=(ko == KO - 1),
                    )
                    nc.tensor.matmul(
                        z1[:], ones_sb[:], p_sb[:, ko, 512:1024],
                        start=(ko == 0), stop=(ko == KO - 1),
                    )
                nc.vector.reciprocal(zrow_sb[0:1, 0:512], z0[:])
                nc.vector.reciprocal(zrow_sb[0:1, 512:1024], z1[:])
                # redistribute [1, 1024] -> [128, QT] via DRAM bounce
                zdram = dram_pool.tile([1, HALF], F32)
                nc.sync.dma_start(zdram[:], zrow_sb[:])
                nc.sync.dma_start(
                    rz_sb[:, :],
                    zdram.rearrange("o (qt p) -> (o p) qt", p=P),
                )

                # T^T[d, q] = X^T @ P
                for dt in range(DO):
                    psT0 = ps_pool.tile([P, 512], F32, tag="ps")
                    psT1 = ps_pool.tile([P, 512], F32, tag="ps")
                    for ko in range(KO):
                        xk_ap = xn_sb[:, ko, dt * P : (dt + 1) * P]
                        nc.tensor.matmul(
                            psT0[:], xk_ap, p_sb[:, ko, 0:512],
                            start=(ko == 0), stop=(ko == KO - 1),
                        )
                        nc.tensor.matmul(
                            psT1[:], xk_ap, p_sb[:, ko, 512:1024],
                            start=(ko == 0), stop=(ko == KO - 1),
                        )
                    nc.vector.tensor_copy(tT_sb[:, dt, 0:512], psT0[:])
                    nc.vector.tensor_copy(tT_sb[:, dt, 512:1024], psT1[:])

                # y = (T^T.T @ Wv) * (1/Z) + bv
                for qt in range(QT):
                    y0 = ps_pool.tile([P, 512], F32, tag="ps")
                    y1 = ps_pool.tile([P, 512], F32, tag="ps")
                    for dt in range(DO):
                        t_ap = tT_sb[:, dt, qt * P : (qt + 1) * P]
                        nc.tensor.matmul(
                            y0[:], t_ap, wv_sb[:, dt, 0:512],
                            start=(dt == 0), stop=(dt == DO - 1),
                        )
                        nc.tensor.matmul(
                            y1[:], t_ap, wv_sb[:, dt, 512:1024],
                            start=(dt == 0), stop=(dt == DO - 1),
                        )
                    o0 = out_pool.tile([P, 512], F32, tag="outp")
                    o1 = out_pool.tile([P, 512], F32, tag="outp")
                    # 1/Z scale on ACT (per-partition scale AP), bv add on DVE
                    nc.scalar.activation(
                        o0[:], y0[:], mybir.ActivationFunctionType.Copy,
                        scale=rz_sb[:, qt : qt + 1],
                    )
                    nc.scalar.activation(
                        o1[:], y1[:], mybir.ActivationFunctionType.Copy,
                        scale=rz_sb[:, qt : qt + 1],
                    )
                    nc.vector.tensor_tensor(
                        o0[:], o0[:], bv_sb[:, 0:512], mybir.AluOpType.add
                    )
                    nc.vector.tensor_tensor(
                        o1[:], o1[:], bv_sb[:, 512:1024], mybir.AluOpType.add
                    )
                    nc.sync.dma_start(y[qt * P : (qt + 1) * P, 0:512], o0[:])
                    nc.sync.dma_start(y[qt * P : (qt + 1) * P, 512:1024], o1[:])

    nc.finalize()
    return nc


_NC_CACHE = None


def make_in_maps(x, Wk, bk, Wq, bq, Wv, bv):
    import ml_dtypes

    bf16 = ml_dtypes.bfloat16
    x = np.asarray(x, dtype=np.float32)

    def _wre(W):
        # [D, D] -> [EO, P(part), DO*P]: each e-tile slice is one
        # fully contiguous per-partition DMA
        W = np.asarray(W, np.float32).reshape(DO, P, EO, P)
        return np.ascontiguousarray(
            W.transpose(2, 1, 0, 3).reshape(EO, P, DO * P).astype(bf16)
        )

    Wk2 = _wre(Wk)
    Wq2 = _wre(Wq)
    Wv2 = np.ascontiguousarray(np.asarray(Wv, np.float32).astype(bf16))
    bkT = np.ascontiguousarray(np.asarray(bk, np.float32).reshape(EO, P).T)
    bqT = np.ascontiguousarray(np.asarray(bq, np.float32).reshape(EO, P).T)
    bv2 = np.ascontiguousarray(
        np.broadcast_to(np.asarray(bv, np.float32).reshape(1, D), (P, D))
    )

    in_maps = []
    for c in range(8):
        b, h = c // 2, c % 2
        xb = x[b]
        if h == 1:
            # swap the s-halves so this core's query half is always first;
            # keys are reindexed consistently (softmax/sum over keys is
            # permutation invariant)
            xb = np.concatenate([xb[HALF:], xb[:HALF]], axis=0)
        xb16 = xb.astype(bf16)
        in_maps.append(
            {
                "xT": np.ascontiguousarray(xb16.T),
                "xN": np.ascontiguousarray(xb16),
                "Wk": Wk2, "Wq": Wq2, "Wv": Wv2,
                "bkT": bkT, "bqT": bqT, "bv": bv2,
            }
        )
    return in_maps


def gather_out(results):
    out = np.empty((B, S, D), dtype=np.float32)
    for c in range(8):
        b, h = c // 2, c % 2
        out[b, h * HALF : (h + 1) * HALF, :] = results[c]["y"]
    return out


def kernel(x, Wk, bk, Wq, bq, Wv, bv):
    global _NC_CACHE
    if _NC_CACHE is None:
        _NC_CACHE = build_nc()
    in_maps = make_in_maps(x, Wk, bk, Wq, bq, Wv, bv)
    res = run_bass_kernel_spmd(_NC_CACHE, in_maps, list(range(8)))
    return gather_out(res.results)
